# revision 22
# baseline (speedup 1.0000x reference)
"""Trainium2 Bass kernel for nn_CompleteAttention_68418829025814.

Linformer-style windowed attention, restructured for the PE array:
  - window_reverse is folded into a host-side column permutation of E_w/F_w
    (device works entirely in x's native window order) and a host-side
    permutation of the gathered output.
  - k/v are never materialized: k_low = (E @ x) @ Wk^T + const (the E/F
    projections contract over tokens, so x is used in its native layout).
  - q is never materialized either: scores_h = (k_low_h^T Wq_h scale) @ x^T
    = M_h @ x^T, with the tiny per-batch M_h = [c, r] built on device and
    the q bias folded into the exp's per-partition bias vector.
  - phase A contracts all 4 batches at once ([128, 768] x-chunks, double
    chunks per DMA on the otherwise-idle scalar queue), so k_low/v_low/M
    for every batch are ready ~25us in and phase B covers the rest.
  - phase B per-tile emission is ordered for the in-order engine queues:
    scores+exps(t) -> back(t-1) (recip/divide/proj) -> av/z(t), keeping the
    PE fed (proj of t-1) while ACT runs t's exps.
  - output projection runs with proj_w stationary; outputs land feature-major
    [c_out, token] (psum banks aliased onto the consumed avA/zA tiles) and
    the final transpose happens on host.
  - HW constraints found the hard way: PSUM tensors must be single-bank, and
    engine reads of PSUM must start at partition 0.

Sharding: data-parallel over batch; each of the 8 cores gets 4 batches
(256 windows) of x. Small weights are replicated.
"""

import numpy as np

B_TOT = 32
N_CORES = 8
B_PER = B_TOT // N_CORES      # 4 batches per core
N = 3136                      # tokens per batch
NP = 3200                     # padded tokens per batch (6*512 + 128)
C = 192
H = 6
HD = 32
R = 128
WS = 7

_STATE = {}


def _window_perm():
    """n_of_m[m] = spatial index n for window-order position m."""
    hh, ww, i, j = np.meshgrid(
        np.arange(8), np.arange(8), np.arange(7), np.arange(7), indexing="ij"
    )
    m = (hh * 8 + ww) * 49 + i * 7 + j
    n = (hh * 7 + i) * 56 + ww * 7 + j
    n_of_m = np.empty(N, dtype=np.int64)
    n_of_m[m.ravel()] = n.ravel()
    return n_of_m


def _build_bass():
    import concourse.bacc as bacc
    import concourse.mybir as mybir
    from concourse.tile import TileContext

    f32 = mybir.dt.float32
    f16 = mybir.dt.float16

    nc = bacc.Bacc("TRN2", target_bir_lowering=False, debug=False)

    # x_a: phase-A layout, row n = [x[0, n, :] | x[1, n, :] | x[2, n, :] | x[3, n, :]]
    x_d = nc.dram_tensor("x_a", [NP, 4 * C], f16, kind="ExternalInput")
    # x^T for the scores: rows = channels (hi 0:128; lo = 128:192 duplicated
    # onto partitions 0-63 and 64-127 for row-band pairing)
    xth_d = nc.dram_tensor("xt_hi", [128, B_PER * NP], f16, kind="ExternalInput")
    xtl_d = nc.dram_tensor("xt_lo2", [128, B_PER * NP], f16, kind="ExternalInput")
    # e/f shipped pre-chunked: row p = 24 chunks of 128 R-values (token 128k+p)
    e_d = nc.dram_tensor("e_wxt", [128, 24 * R], f16, kind="ExternalInput")
    f_d = nc.dram_tensor("f_wxt", [128, 24 * R], f16, kind="ExternalInput")
    e_tl_d = nc.dram_tensor("e_tl", [64, R], f16, kind="ExternalInput")
    f_tl_d = nc.dram_tensor("f_tl", [64, R], f16, kind="ExternalInput")
    wkt_d = nc.dram_tensor("wkt", [C, C], f16, kind="ExternalInput")
    wvt_d = nc.dram_tensor("wvt", [C, C], f16, kind="ExternalInput")
    ckt_d = nc.dram_tensor("const_kt", [C, R], f32, kind="ExternalInput")
    cv_d = nc.dram_tensor("const_v", [R, C], f32, kind="ExternalInput")
    # Wq (scaled) laid out by q-channel rows; used to build M_h on device
    wqh_d = nc.dram_tensor("wq_hi_dc", [128, C], f16, kind="ExternalInput")
    wql_d = nc.dram_tensor("wq_lo_dc", [64, C], f16, kind="ExternalInput")
    # block-diagonal q-bias columns for the exp bias build
    bqh_d = nc.dram_tensor("bqblk_hi", [128, 4], f16, kind="ExternalInput")
    bql_d = nc.dram_tensor("bqblk_lo", [64, 2], f16, kind="ExternalInput")
    # proj weights, stationary chunks: pw = proj_w.T (ch, co)
    pwhh_d = nc.dram_tensor("pw_hh", [128, 128], f16, kind="ExternalInput")
    pwhl_d = nc.dram_tensor("pw_hl", [128, 64], f16, kind="ExternalInput")
    pwlh_d = nc.dram_tensor("pw_lh_aug", [65, 128], f16, kind="ExternalInput")
    pwll_d = nc.dram_tensor("pw_ll_aug", [65, 64], f16, kind="ExternalInput")
    ident_d = nc.dram_tensor("ident", [128, 128], f16, kind="ExternalInput")
    ones_d = nc.dram_tensor("ones_att", [128, 32], f16, kind="ExternalInput")
    onesrow_d = nc.dram_tensor("ones_row", [1, 512], f16, kind="ExternalInput")
    # outputs feature-major: out[c, b*NP + n]
    outh_d = nc.dram_tensor("out_hi", [128, B_PER * NP], f16, kind="ExternalOutput")
    outl_d = nc.dram_tensor("out_lo", [64, B_PER * NP], f16, kind="ExternalOutput")

    NCH = 25  # n-chunks per batch for the E/F contraction (24*128 + 64)

    with TileContext(nc) as tc:
        with tc.tile_pool(name="const", bufs=1) as cpool, \
             tc.tile_pool(name="ef", bufs=1) as efpool, \
             tc.tile_pool(name="low", bufs=1) as lowpool, \
             tc.tile_pool(name="mh", bufs=1) as mhpool, \
             tc.tile_pool(name="xin", bufs=4) as xpool, \
             tc.tile_pool(name="xt", bufs=3) as xtpool, \
             tc.tile_pool(name="sp", bufs=3) as sppool, \
             tc.tile_pool(name="div", bufs=2) as divpool, \
             tc.tile_pool(name="av", bufs=2) as avpool, \
             tc.tile_pool(name="osb", bufs=2) as opool, \
             tc.tile_pool(name="ps", bufs=4, space="PSUM") as ps:

            # ---- constants (sync queue; scalar queue is reserved for the
            # phase-A x stream, gpsimd for e/f) ----
            ident = cpool.tile([128, 128], f16)
            nc.sync.dma_start(ident[:], ident_d[:])
            wkt = cpool.tile([128, C], f16)
            nc.sync.dma_start(wkt[:], wkt_d[0:128, :])
            wkt_l = cpool.tile([64, C], f16)
            nc.sync.dma_start(wkt_l[:], wkt_d[128:192, :])
            wvt = cpool.tile([128, C], f16)
            nc.sync.dma_start(wvt[:], wvt_d[0:128, :])
            wvt_l = cpool.tile([64, C], f16)
            nc.sync.dma_start(wvt_l[:], wvt_d[128:192, :])
            ckt_h = cpool.tile([128, R], f32)
            nc.sync.dma_start(ckt_h[:], ckt_d[0:128, :])
            ckt_l = cpool.tile([64, R], f32)
            nc.sync.dma_start(ckt_l[:], ckt_d[128:192, :])
            cv = cpool.tile([128, C], f32)
            nc.sync.dma_start(cv[:], cv_d[:])
            wq_hi = cpool.tile([128, C], f16)
            nc.sync.dma_start(wq_hi[:], wqh_d[:])
            wq_lo = cpool.tile([64, C], f16)
            nc.sync.dma_start(wq_lo[:], wql_d[:])
            bq_hi = cpool.tile([128, 4], f16)
            nc.sync.dma_start(bq_hi[:], bqh_d[:])
            bq_lo = cpool.tile([64, 2], f16)
            nc.sync.dma_start(bq_lo[:], bql_d[:])
            pw_hh = cpool.tile([128, 128], f16)
            nc.sync.dma_start(pw_hh[:], pwhh_d[:])
            pw_hl = cpool.tile([128, 64], f16)
            nc.sync.dma_start(pw_hl[:], pwhl_d[:])
            pw_lh = cpool.tile([65, 128], f16)
            nc.sync.dma_start(pw_lh[:], pwlh_d[:])
            pw_ll = cpool.tile([65, 64], f16)
            nc.sync.dma_start(pw_ll[:], pwll_d[:])
            ones_att = cpool.tile([128, 32], f16)
            nc.sync.dma_start(ones_att[:], ones_d[:])
            # persistent [65, 512] attn-output staging tiles whose row 64
            # stays 1.0 forever (feeds proj_b through pw_l*_aug's last row)
            av_lo_bufs = [cpool.tile([65, 512], f16, name=f"avlo{i}") for i in range(2)]
            for i in range(2):
                nc.sync.dma_start(av_lo_bufs[i][64:65, :], onesrow_d[:])

            # E/F transposed weights resident in SBUF: 24 full chunks + tail
            e_sb = efpool.tile([128, 24, 128], f16)
            f_sb = efpool.tile([128, 24, 128], f16)
            e_tl = efpool.tile([64, 128], f16)
            f_tl = efpool.tile([64, 128], f16)

            def load_ef_group(g):
                sl = slice(g * 6, (g + 1) * 6)
                dsl = slice(g * 6 * 128, (g + 1) * 6 * 128)
                nc.gpsimd.dma_start(
                    e_sb[:, sl, :].rearrange("p k r -> p (k r)"), e_d[:, dsl]
                )
                nc.gpsimd.dma_start(
                    f_sb[:, sl, :].rearrange("p k r -> p (k r)"), f_d[:, dsl]
                )

            for g in range(4):
                load_ef_group(g)
            nc.gpsimd.dma_start(e_tl[:], e_tl_d[:])
            nc.gpsimd.dma_start(f_tl[:], f_tl_d[:])

            # per-batch low-rank tensors (kept resident across phase B)
            klo_h = [lowpool.tile([128, R], f16, name=f"klo_h{b}") for b in range(B_PER)]
            klo_l = [lowpool.tile([64, R], f16, name=f"klo_l{b}") for b in range(B_PER)]
            vlo = [lowpool.tile([128, C], f16, name=f"vlo{b}") for b in range(B_PER)]

            # ---------------- Phase A: EP/FP for all 4 batches ----------------
            # x chunks are [nk, 768] = 4 batches wide; two chunks per DMA on
            # the dedicated scalar queue.
            ep_a = ps.tile([128, 2 * C], f32, name="ep_a", tag="bk", bufs=4)
            ep_b = ps.tile([128, 2 * C], f32, name="ep_b", tag="bk", bufs=4)
            fp_a = ps.tile([128, 2 * C], f32, name="fp_a", tag="bk", bufs=4)
            fp_b = ps.tile([128, 2 * C], f32, name="fp_b", tag="bk", bufs=4)
            for cp in range(13):
                k = 2 if cp < 12 else 1
                x2 = xpool.tile([128, 2, 4 * C], f16, name="x2", tag="x2")
                nc.scalar.dma_start(
                    x2[:, 0:k, :],
                    x_d[cp * 256 : cp * 256 + k * 128, :].rearrange(
                        "(k p) c -> p k c", p=128
                    ),
                )
                for kk in range(k):
                    ci = cp * 2 + kk
                    nk = 128 if ci < 24 else 64
                    elh = e_sb[:, ci, :] if ci < 24 else e_tl[:]
                    flh = f_sb[:, ci, :] if ci < 24 else f_tl[:]
                    x2f = x2[0:nk, kk, :]
                    st = ci == 0
                    sp_ = ci == NCH - 1
                    nc.tensor.matmul(
                        ep_a[:], elh, x2f[:, 0 : 2 * C], start=st, stop=sp_
                    )
                    nc.tensor.matmul(
                        fp_a[:], flh, x2f[:, 0 : 2 * C], start=st, stop=sp_
                    )
                    nc.tensor.matmul(
                        ep_b[:], elh, x2f[:, 2 * C : 4 * C], start=st, stop=sp_
                    )
                    nc.tensor.matmul(
                        fp_b[:], flh, x2f[:, 2 * C : 4 * C], start=st, stop=sp_
                    )
            ep_sb = xpool.tile([128, 4 * C], f16, name="ep_sb", tag="ep_sb")
            nc.vector.tensor_copy(ep_sb[:, 0 : 2 * C], ep_a[:])
            nc.vector.tensor_copy(ep_sb[:, 2 * C : 4 * C], ep_b[:])
            fp_sb = xpool.tile([128, 4 * C], f16, name="fp_sb", tag="fp_sb")
            nc.vector.tensor_copy(fp_sb[:, 0 : 2 * C], fp_a[:])
            nc.vector.tensor_copy(fp_sb[:, 2 * C : 4 * C], fp_b[:])

            # per-batch M_h = [c, r] score matrices + exp bias vectors
            mh_hi = [mhpool.tile([128, 6, 128], f16, name=f"mh_hi{b}")
                     for b in range(B_PER)]
            mh_lo = [mhpool.tile([128, 6, 128], f16, name=f"mh_lo{b}")
                     for b in range(B_PER)]
            cb_sb = [mhpool.tile([128, 8], f32, name=f"cb{b}")
                     for b in range(B_PER)]

            def lowrank(b):
                # transpose EP, FP slices: (r=128, c=192) -> (c, r)
                ept_h = xpool.tile([128, 128], f16, name="ept_h", tag="ept_h")
                ept_l = xpool.tile([64, 128], f16, name="ept_l", tag="ept_l")
                fpt_h = xpool.tile([128, 128], f16, name="fpt_h", tag="fpt_h")
                fpt_l = xpool.tile([64, 128], f16, name="fpt_l", tag="fpt_l")
                for (src, dsth, dstl) in ((ep_sb, ept_h, ept_l), (fp_sb, fpt_h, fpt_l)):
                    tp1 = ps.tile([128, 128], f16, name="tp1", tag="sb", bufs=4)
                    nc.tensor.transpose(
                        tp1[:], src[:, b * C : b * C + 128], ident[:]
                    )
                    nc.vector.tensor_copy(dsth[:], tp1[:])
                    tp2 = ps.tile([64, 128], f16, name="tp2", tag="sb", bufs=4)
                    nc.tensor.transpose(
                        tp2[:], src[:, b * C + 128 : b * C + 192], ident[:]
                    )
                    nc.vector.tensor_copy(dstl[:], tp2[:])

                # k_lowT = WkT.T @ EPT + const_kT  (feature-major (kch, r))
                kl_hi = ps.tile([128, R], f32, name="kl_hi", tag="sb", bufs=4)
                nc.tensor.matmul(kl_hi[:], wkt[:, 0:128], ept_h[:], start=True, stop=False)
                nc.tensor.matmul(kl_hi[:], wkt_l[:, 0:128], ept_l[:], start=False, stop=True)
                nc.vector.tensor_tensor(
                    klo_h[b][:], kl_hi[:], ckt_h[:], op=mybir.AluOpType.add
                )
                kl_lo = ps.tile([64, R], f32, name="kl_lo", tag="sb", bufs=4)
                nc.tensor.matmul(kl_lo[:], wkt[:, 128:192], ept_h[:], start=True, stop=False)
                nc.tensor.matmul(kl_lo[:], wkt_l[:, 128:192], ept_l[:], start=False, stop=True)
                nc.vector.tensor_tensor(
                    klo_l[b][:], kl_lo[:], ckt_l[:], op=mybir.AluOpType.add
                )
                # v_low (R-major (r, vch)), straight to f16 with const add
                vl_ps = ps.tile([128, C], f32, name="vl_ps", tag="sb", bufs=4)
                nc.tensor.matmul(vl_ps[:], fpt_h[:], wvt[:], start=True, stop=False)
                nc.tensor.matmul(vl_ps[:], fpt_l[:], wvt_l[:], start=False, stop=True)
                nc.vector.tensor_tensor(
                    vlo[b][:], vl_ps[:], cv[:], op=mybir.AluOpType.add
                )

            lowrank(0)

            # M_h = klo_h^T @ Wq_h (both indexed by q-channel rows), laid
            # out [c, r]; the c 128:192 part is duplicated onto partitions
            # 64-127 so the lo score matmuls can run band-paired. Also the
            # exp bias cb[r, h] = sum_d klo[d, r] * bq_scaled[d].
            def build_mh(b, h):
                if h < 4:
                    kslice = klo_h[b][32 * h : 32 * h + 32, :]
                    wslice_hi = wq_hi[32 * h : 32 * h + 32, 0:128]
                    wslice_lo = wq_hi[32 * h : 32 * h + 32, 128:192]
                    tp = (32 * h, 0)
                else:
                    hh = h - 4
                    kslice = klo_l[b][32 * hh : 32 * hh + 32, :]
                    wslice_hi = wq_lo[32 * hh : 32 * hh + 32, 0:128]
                    wslice_lo = wq_lo[32 * hh : 32 * hh + 32, 128:192]
                    tp = (32 * hh, 0)
                m1 = ps.tile([128, 128], f32, name="m1", tag="sb", bufs=4)
                nc.tensor.matmul(m1[:], wslice_hi, kslice, start=True,
                                 stop=True, tile_position=tp)
                nc.vector.tensor_copy(mh_hi[b][:, h, :], m1[:])
                m2 = ps.tile([128, 128], f32, name="m2", tag="sb", bufs=4)
                nc.tensor.matmul(m2[0:64, :], wslice_lo, kslice, start=True,
                                 stop=True, tile_position=(tp[0], 0))
                nc.tensor.matmul(m2[64:128, :], wslice_lo, kslice, start=True,
                                 stop=True, tile_position=(tp[0], 64))
                nc.vector.tensor_copy(mh_lo[b][:, h, :], m2[:])

            def build_cb(b):
                cbp = ps.tile([128, 8], f32, name="cbp", tag="sb", bufs=4)
                nc.tensor.matmul(cbp[:, 0:4], klo_h[b][:], bq_hi[:],
                                 start=True, stop=True)
                nc.tensor.matmul(cbp[:, 4:6], klo_l[b][:], bq_lo[:],
                                 start=True, stop=True)
                nc.vector.tensor_copy(cb_sb[b][:, 0:6], cbp[:, 0:6])

            for h in range(6):
                build_mh(0, h)
            build_cb(0)

            # ---------------- Phase B: attention tiles ----------
            def front1(b, t):
                """x^T loads + scores + exps for tile (b, t)"""
                W = 512 if t < 6 else 128
                base = b * NP + t * 512
                xth = xtpool.tile([128, W], f16, name="xth", tag="xth")
                nc.sync.dma_start(xth[:], xth_d[:, base : base + W])
                xtl = xtpool.tile([128, W], f16, name="xtl", tag="xtl")
                nc.sync.dma_start(xtl[:], xtl_d[:, base : base + W])
                sps = []
                sbanks = []
                for h in range(6):
                    s1 = ps.tile([128, W], f32, name=f"s{h}", tag="sb", bufs=4)
                    nc.tensor.matmul(
                        s1[:], mh_hi[b][:, h, :], xth[:],
                        start=True, stop=False,
                    )
                    sbanks.append(s1)
                    if h % 2 == 1:
                        for j, hp in enumerate((h - 1, h)):
                            nc.tensor.matmul(
                                sbanks[hp][:],
                                mh_lo[b][64 * j : 64 * j + 64, hp, :],
                                xtl[64 * j : 64 * j + 64, :],
                                start=False, stop=True,
                                tile_position=(64 * j, 0),
                            )
                        for hp in (h - 1, h):
                            sp1 = sppool.tile(
                                [128, W], f16, name=f"sp{hp}", tag=f"sp{hp}"
                            )
                            nc.scalar.activation(
                                sp1[:], sbanks[hp][:],
                                mybir.ActivationFunctionType.Exp,
                                bias=cb_sb[b][:, hp : hp + 1],
                            )
                            sps.append(sp1)
                return dict(W=W, base=base, b=b, t=t, sps=sps)

            def front2(st):
                """attn @ v_low + denominators for tile (b, t)"""
                W, b, t, sps = st["W"], st["b"], st["t"], st["sps"]
                avA = ps.tile([128, W], f32, name="avA", tag="bk", bufs=4)
                for h in range(4):
                    nc.tensor.matmul(
                        avA[32 * h : 32 * h + 32, :],
                        vlo[b][:, 32 * h : 32 * h + 32],
                        sps[h][:],
                        start=True, stop=True,
                        tile_position=(0, 32 * h),
                    )
                zA = ps.tile([128, W], f32, name="zA", tag="bk", bufs=4)
                for h in range(4):
                    nc.tensor.matmul(
                        zA[32 * h : 32 * h + 32, :],
                        ones_att[:],
                        sps[h][:],
                        start=True, stop=True,
                        tile_position=(0, 32 * h),
                    )
                av2 = ps.tile([64, W], f32, name="av2", tag="bk", bufs=4)
                for hh in range(2):
                    nc.tensor.matmul(
                        av2[32 * hh : 32 * hh + 32, :],
                        vlo[b][:, 128 + 32 * hh : 160 + 32 * hh],
                        sps[4 + hh][:],
                        start=True, stop=True,
                        tile_position=(0, 32 * hh),
                    )
                z2 = ps.tile([64, W], f32, name="z2", tag="bk", bufs=4)
                for hh in range(2):
                    nc.tensor.matmul(
                        z2[32 * hh : 32 * hh + 32, :],
                        ones_att[:],
                        sps[4 + hh][:],
                        start=True, stop=True,
                        tile_position=(0, 32 * hh),
                    )
                av_hi = avpool.tile([128, W], f16, name="av_hi", tag="av_hi")
                av_lo = av_lo_bufs[(b * 7 + t) % 2]
                st.update(avA=avA, zA=zA, av2=av2, z2=z2, av_hi=av_hi, av_lo=av_lo)

            osb_state = {}

            def back(st):
                W, base, t = st["W"], st["base"], st["t"]
                rzA = divpool.tile([128, W], f32, name="rzA", tag="rzA")
                nc.vector.reciprocal_approx_fast(rzA[:], st["zA"][:])
                rz2 = divpool.tile([64, W], f32, name="rz2", tag="rz2")
                nc.vector.reciprocal_approx_fast(rz2[:], st["z2"][:])
                av_hi, av_lo = st["av_hi"], st["av_lo"]
                nc.vector.tensor_tensor(
                    av_hi[:, :], st["avA"][:], rzA[:], op=mybir.AluOpType.mult
                )
                nc.vector.tensor_tensor(
                    av_lo[0:64, 0:W], st["av2"][:], rz2[:], op=mybir.AluOpType.mult
                )
                # output projection: proj weights stationary, av moving; the
                # psum outputs alias the (already consumed) avA / zA banks.
                o1 = st["avA"]
                nc.tensor.matmul(o1[:], pw_hh[:], av_hi[:, 0:W], start=True, stop=False)
                nc.tensor.matmul(o1[:], pw_lh[:], av_lo[:, 0:W], start=False, stop=True)
                o2 = st["zA"][0:64, :]
                nc.tensor.matmul(o2, pw_hl[:], av_hi[:, 0:W], start=True, stop=False)
                nc.tensor.matmul(o2, pw_ll[:], av_lo[:, 0:W], start=False, stop=True)
                # stage pairs of tiles and DMA once per pair; the hi copy runs
                # on ACT, the lo copy on DVE (engine balance)
                if t % 2 == 0:
                    osb_state["hi"] = opool.tile([128, 1024], f16, name="osb", tag="osb")
                    osb_state["lo"] = opool.tile([64, 1024], f16, name="osb2", tag="osb2")
                    osb_state["base"] = base
                off = (t % 2) * 512
                osb, osb2 = osb_state["hi"], osb_state["lo"]
                nc.vector.tensor_copy(osb[:, off : off + W], o1[:])
                nc.vector.tensor_copy(osb2[:, off : off + W], o2)
                if t % 2 == 1 or t == 6:
                    w_tot = 512 + W if t % 2 == 1 else W
                    b0 = osb_state["base"]
                    nc.gpsimd.dma_start(
                        outh_d[:, b0 : b0 + w_tot], osb[:, 0:w_tot]
                    )
                    nc.gpsimd.dma_start(
                        outl_d[:, b0 : b0 + w_tot], osb2[:, 0:w_tot]
                    )

            from functools import partial
            weave = {
                (0, 1): [partial(lowrank, 1)],
                (0, 2): [partial(build_mh, 1, 0), partial(build_mh, 1, 1)],
                (0, 3): [partial(build_mh, 1, 2), partial(build_mh, 1, 3)],
                (0, 4): [partial(build_mh, 1, 4), partial(build_mh, 1, 5)],
                (0, 5): [partial(build_cb, 1), partial(lowrank, 2)],
                (0, 6): [partial(build_mh, 2, 0), partial(build_mh, 2, 1)],
                (1, 0): [partial(build_mh, 2, 2), partial(build_mh, 2, 3)],
                (1, 1): [partial(build_mh, 2, 4), partial(build_mh, 2, 5)],
                (1, 2): [partial(build_cb, 2), partial(lowrank, 3)],
                (1, 3): [partial(build_mh, 3, 0), partial(build_mh, 3, 1)],
                (1, 4): [partial(build_mh, 3, 2), partial(build_mh, 3, 3)],
                (1, 5): [partial(build_mh, 3, 4), partial(build_mh, 3, 5)],
                (1, 6): [partial(build_cb, 3)],
            }
            tiles = [(b, t) for b in range(B_PER) for t in range(7)]
            prev = None
            for b, t in tiles:
                st = front1(b, t)
                if prev is not None:
                    back(prev)
                # weave the remaining lowrank / M_h / cb builds (batches 1-3)
                # over early tiles, each finishing before its batch starts
                for job in weave.get((b, t), ()):
                    job()
                front2(st)
                prev = st
            back(prev)

    nc.compile()
    return nc


def _get_nc():
    if "nc" not in _STATE:
        _STATE["nc"] = _build_bass()
    return _STATE["nc"]


def kernel(x, qkv_w, qkv_b, E_w, E_b, F_w, F_b, proj_w, proj_b, h, w):
    from concourse.bass_utils import run_bass_kernel_spmd

    x = np.asarray(x, dtype=np.float32)
    qkv_w = np.asarray(qkv_w, dtype=np.float32)
    qkv_b = np.asarray(qkv_b, dtype=np.float32)
    E_w = np.asarray(E_w, dtype=np.float32)
    E_b = np.asarray(E_b, dtype=np.float32)
    F_w = np.asarray(F_w, dtype=np.float32)
    F_b = np.asarray(F_b, dtype=np.float32)
    proj_w = np.asarray(proj_w, dtype=np.float32)
    proj_b = np.asarray(proj_b, dtype=np.float32)
    assert int(h) == 56 and int(w) == 56

    n_of_m = _window_perm()
    E_wx = np.ascontiguousarray(E_w[:, n_of_m])
    F_wx = np.ascontiguousarray(F_w[:, n_of_m])

    Wq, Wk, Wv = qkv_w[0:C], qkv_w[C : 2 * C], qkv_w[2 * C : 3 * C]
    bq, bk, bv = qkv_b[0:C], qkv_b[C : 2 * C], qkv_b[2 * C : 3 * C]
    scale = np.float32(1.0 / np.sqrt(HD))

    const_k = np.outer(E_wx.sum(1), bk) + E_b[:, None]      # (128, 192)
    const_v = (np.outer(F_wx.sum(1), bv) + F_b[:, None]).astype(np.float32)

    # Wq rows are q-channels; scaled by 1/sqrt(hd). M_h is built on device.
    wq_hi_dc = np.ascontiguousarray(Wq[0:128, :] * scale).astype(np.float16)
    wq_lo_dc = np.ascontiguousarray(Wq[128:192, :] * scale).astype(np.float16)
    bqblk_hi = np.zeros((128, 4), dtype=np.float16)
    for hh in range(4):
        bqblk_hi[32 * hh : 32 * hh + 32, hh] = (bq[32 * hh : 32 * hh + 32] * scale)
    bqblk_lo = np.zeros((64, 2), dtype=np.float16)
    for hh in range(2):
        bqblk_lo[32 * hh : 32 * hh + 32, hh] = (
            bq[128 + 32 * hh : 160 + 32 * hh] * scale
        )

    wkt = np.ascontiguousarray(Wk.T).astype(np.float16)
    wvt = np.ascontiguousarray(Wv.T).astype(np.float16)
    ckt = np.ascontiguousarray(const_k.T.astype(np.float32))  # (192, 128)
    pw = proj_w.T                                            # (ch, co)
    pw_hh = np.ascontiguousarray(pw[0:128, 0:128]).astype(np.float16)
    pw_hl = np.ascontiguousarray(pw[0:128, 128:192]).astype(np.float16)
    pw_lh = np.zeros((65, 128), dtype=np.float16)
    pw_lh[0:64] = pw[128:192, 0:128]
    pw_lh[64] = proj_b[0:128]
    pw_ll = np.zeros((65, 64), dtype=np.float16)
    pw_ll[0:64] = pw[128:192, 128:192]
    pw_ll[64] = proj_b[128:192]

    e_wxt_full = np.ascontiguousarray(E_wx.T).astype(np.float16)  # (3136, 128)
    f_wxt_full = np.ascontiguousarray(F_wx.T).astype(np.float16)
    # pre-chunked layout: (24, 128, R) -> (128, 24*R) so each SBUF partition
    # line is one contiguous DMA descriptor
    e_wxt = np.ascontiguousarray(
        e_wxt_full[0:3072].reshape(24, 128, R).transpose(1, 0, 2).reshape(128, 24 * R)
    )
    f_wxt = np.ascontiguousarray(
        f_wxt_full[0:3072].reshape(24, 128, R).transpose(1, 0, 2).reshape(128, 24 * R)
    )
    e_tl = np.ascontiguousarray(e_wxt_full[3072:3136])
    f_tl = np.ascontiguousarray(f_wxt_full[3072:3136])
    ident = np.eye(128, dtype=np.float16)
    ones_att = np.ones((128, 32), dtype=np.float16)
    ones_row = np.ones((1, 512), dtype=np.float16)

    consts = dict(
        e_wxt=e_wxt, f_wxt=f_wxt, e_tl=e_tl, f_tl=f_tl,
        wq_hi_dc=wq_hi_dc, wq_lo_dc=wq_lo_dc,
        bqblk_hi=bqblk_hi, bqblk_lo=bqblk_lo,
        wkt=wkt, wvt=wvt, const_kt=ckt, const_v=const_v,
        pw_hh=pw_hh, pw_hl=pw_hl, pw_lh_aug=pw_lh, pw_ll_aug=pw_ll,
        ident=ident, ones_att=ones_att, ones_row=ones_row,
    )

    # shard x: core i gets batches 4i..4i+4, padded to NP tokens per batch
    xb = x.reshape(B_TOT, 64 * 49, C).astype(np.float16)
    in_maps = []
    for i in range(N_CORES):
        xi = np.zeros((B_PER, NP, C), dtype=np.float16)
        xi[:, 0:N, :] = xb[B_PER * i : B_PER * (i + 1)]
        # phase-A layout: row n = 4 batches' channels side by side
        xa = np.ascontiguousarray(xi.transpose(1, 0, 2).reshape(NP, 4 * C))
        M = B_PER * NP
        xt = xi.reshape(M, C).T                               # (c, M)
        xt_hi = np.ascontiguousarray(xt[0:128])
        xt_lo2 = np.ascontiguousarray(
            np.concatenate([xt[128:192], xt[128:192]], axis=0)
        )
        in_maps.append({**consts, "x_a": xa, "xt_hi": xt_hi, "xt_lo2": xt_lo2})

    nc = _get_nc()
    _STATE["last_in_maps"] = in_maps
    res = run_bass_kernel_spmd(nc, in_maps, core_ids=list(range(N_CORES)))

    out_win = np.empty((B_TOT, N, C), dtype=np.float32)
    for i in range(N_CORES):
        oh = res.results[i]["out_hi"].astype(np.float32)      # (128, B_PER*NP)
        ol = res.results[i]["out_lo"].astype(np.float32)      # (64, B_PER*NP)
        oc = np.concatenate([oh, ol], axis=0)                 # (192, B_PER*NP)
        oi = oc.reshape(C, B_PER, NP).transpose(1, 2, 0)      # (B_PER, NP, C)
        out_win[B_PER * i : B_PER * (i + 1)] = oi[:, 0:N, :]
    # window_reverse on the gathered output
    out_sp = (
        out_win.reshape(B_TOT, 8, 8, 7, 7, C)
        .transpose(0, 1, 3, 2, 4, 5)
        .reshape(B_TOT, N, C)
    )
    return np.ascontiguousarray(out_sp)
